# revision 12
# baseline (speedup 1.0000x reference)
"""Causal self-attention (B=2, S=2048, E=2048, H=16, rope) on 8 TRN2 NeuronCores.

Sharding: tensor-parallel over heads. Each core owns 2 heads (both batches):
w_qkv rows / w_out columns for its heads; every core reads the full x
(replicated, bf16, pre-transposed) and produces a partial [B*S, E] bf16
output; the host sums the 8 partials in f32 (the "all-reduce").

Per-core kernel — a single software-pipelined emission stream:
  - xT [E, B*S] bf16 serves as matmul rhs (Q/K projections -> QT/KT arrive
    transposed [D, S], the layout attention wants) and as lhsT (V
    projection, natural [S, D]).
  - scores are computed transposed: scoresT[k,q] = KT^T @ QT, in panels of
    512 q columns. exp runs on ScalarE (softmax scale folded into the
    activation scale); causal masking = per-kb column offsets + one bf16
    0/1 mask multiply on the diagonal block; A@V and the sums matmuls
    accumulate only each k-block's causally-valid column range.
  - softmax sums over k (partition dim) use a ones[128,128] matmul that
    produces the column sums already broadcast across all 128 partitions;
    reciprocal + multiply fold normalization into the y^T PSUM evacuation.
  - attention emits with a one-block lookahead (scores(kb+1) issues before
    A@V(kb)/sums(kb)) so the PE never sits behind ScalarE's exp.
  - the whole kernel is one interleaved stream: attention panels run in
    FORWARD order (panel p only needs proj token-blocks <= p), and the PE
    gaps left by exp/DVE latency are filled with proj chains of the next
    token block and out-proj chains of already-finished panels.
  - rope is applied on DVE during QKV-PSUM evacuation with [D, S] cos /
    signed-sin tables; the half-rotation uses a partition-rolled sin table
    so both multiplies are full-width.
  - out-proj PSUM is evacuated by DVE to bf16 and DMA'd out (ScalarE is
    reserved for exp).

PSUM budget (8 banks of [128,512]f32):
  pp(2) proj chains | ps(2) attn scores | yps(2) | sps(1) | ops(1) out-proj.
  Out-proj chains alternate ops/pp when projection isn't concurrently
  active (so the final out-proj tail double-buffers).
"""

import math
from collections import deque

import numpy as np
import ml_dtypes

import concourse.bass as bass
import concourse.mybir as mybir
import concourse.tile as tile
from concourse import bacc
from concourse.bass_utils import run_bass_kernel_spmd

B, S, E, H, D = 2, 2048, 2048, 16, 128
NCORES = 8
HL = H // NCORES            # heads per core = 2
NTOK = B * S                # 4096
KE = E // 128               # 16 contraction chunks
NB = S // 128               # 16 k/token blocks per batch
NPANEL = S // 512           # 4 q panels per batch
SOFTMAX_SCALE = 1.0 / math.sqrt(D)
BF16 = mybir.dt.bfloat16
F32 = mybir.dt.float32

ROPE_BASE = 10000.0


def _rope_tables():
    inv_freq = 1.0 / (ROPE_BASE ** (np.arange(0, D, 2, dtype=np.float32) / D))
    pos = np.arange(S, dtype=np.float32)
    freqs = np.outer(pos, inv_freq)               # [S, D/2]
    emb = np.concatenate([freqs, freqs], -1)      # [S, D]
    cosT = np.cos(emb).T.astype(np.float32)       # [D, S]
    sinT = np.sin(emb).T.astype(np.float32)
    sinS = sinT.copy()
    sinS[: D // 2] *= -1.0                        # signed: rotate_half sign folded in
    return np.ascontiguousarray(cosT), np.ascontiguousarray(sinS)


def _emit(nc, tc, xT, wqkvT, w_outT, out, cos_d, sin_d, mask_d):
    from contextlib import ExitStack

    ctx = ExitStack()
    with ctx:
        singles = ctx.enter_context(tc.tile_pool(name="singles", bufs=1))
        xpool = ctx.enter_context(tc.tile_pool(name="xcol", bufs=2))
        persist = ctx.enter_context(tc.tile_pool(name="persist", bufs=1))
        ropet = ctx.enter_context(tc.tile_pool(name="ropet", bufs=3))
        attnp = ctx.enter_context(tc.tile_pool(name="attn", bufs=12))
        evacp = ctx.enter_context(tc.tile_pool(name="evac", bufs=2))
        outp = ctx.enter_context(tc.tile_pool(name="outp", bufs=4))
        psum = ctx.enter_context(tc.tile_pool(name="psum", bufs=2, space="PSUM"))

        # ---- constant tiles (bulky ones ride the SWDGE queues so the HWDGE
        # queues carry only the latency-critical wq/xc stream) ----
        wq_sb = [singles.tile([128, 3 * HL * D], BF16, tag=f"wq{ke}", name=f"wq{ke}")
                 for ke in range(KE)]
        wo_sb = singles.tile([128, HL, E], BF16, tag="wo")
        cos_sb = singles.tile([128, S], F32, tag="cos")
        sin_sb = singles.tile([128, S], F32, tag="sin")
        mask_sb = singles.tile([128, 128], BF16, tag="mask")
        ones_kk = singles.tile([128, 128], BF16, tag="oneskk")
        nc.vector.memset(ones_kk, 1.0)
        # ~48 one-column dummy matmuls: keep the PE's HAM activity monitor
        # busy while the first x/weight DMAs land, so real matmuls start at
        # the warm 2.4GHz clock instead of paying the 1.2GHz cold ramp
        warm = psum.tile([128, 512], F32, tag="pp", bufs=2, name="warm")
        for _ in range(48):
            nc.tensor.matmul(warm[:, 0:1], lhsT=ones_kk, rhs=ones_kk[:, 0:1],
                             start=True, stop=True)
        nc.gpsimd.dma_start(out=cos_sb, in_=cos_d)
        nc.gpsimd.dma_start(out=sin_sb, in_=sin_d)
        nc.gpsimd.dma_start(out=mask_sb, in_=mask_d)
        for hl in range(HL):
            nc.gpsimd.dma_start(
                out=wo_sb[:, hl, :], in_=w_outT[hl * 128:(hl + 1) * 128, :]
            )

        # ---- persistent per-(b,h) tensors ----
        q_sb = [[persist.tile([128, S], BF16, tag=f"q{b}{h}", name=f"q{b}{h}") for h in range(HL)] for b in range(B)]
        k_sb = [[persist.tile([128, S], BF16, tag=f"k{b}{h}", name=f"k{b}{h}") for h in range(HL)] for b in range(B)]
        v_sb = [persist.tile([128, NB, HL * D], BF16, tag=f"v{b}", name=f"v{b}") for b in range(B)]
        y_sb = [[persist.tile([128, S], BF16, tag=f"y{b}{h}", name=f"y{b}{h}") for h in range(HL)] for b in range(B)]

        # ============ emission generators (each yield ~ one PE work unit) ====

        xc_map = {}

        def open_tb(b, sb4):
            """Issue the 16 x-chunk DMAs for one token block. Alternates the
            Sync and Scalar HWDGE queues so two descriptor generators pipeline
            in parallel (a single HWDGE queue paces at ~2.2us per [128,*]
            transfer front-to-back). Called early so data is resident before
            the matmuls need it."""
            tb = b * (S // 512) + sb4
            xc = []
            for ke in range(KE):
                x1 = xpool.tile([128, 512], BF16, tag=f"xc{ke}", name=f"xc{tb}_{ke}")
                if tb == 0:
                    # weights ride Sync while x rides Scalar: both descriptor
                    # generators run from the first instruction
                    nc.sync.dma_start(
                        out=wq_sb[ke], in_=wqkvT[ke * 128:(ke + 1) * 128, :]
                    )
                    xeng = nc.scalar
                else:
                    xeng = nc.sync if ke % 2 == 0 else nc.scalar
                xeng.dma_start(
                    out=x1,
                    in_=xT[ke * 128:(ke + 1) * 128, tb * 512:(tb + 1) * 512],
                )
                xc.append(x1)
            xc_map[tb] = xc

        def proj_tb(b, sb4):
            """QKV projection of one 512-token column block. Yields after each
            contraction step (~215-430ns PE). 8 chains: 4 QK rows + 4 V
            token-blocks, accumulating in pp-tag PSUM. The first block borrows
            the idle attention PSUM banks so 6 chains can consume the arriving
            DMA stream in parallel."""
            tb = b * (S // 512) + sb4
            soff = sb4 * 512
            if tb not in xc_map:
                open_tb(b, sb4)
            xc = xc_map.pop(tb)
            chains = [("qk", rb) for rb in range(2 * HL)] + [
                ("v", tsb) for tsb in range(4)
            ]
            if tb == 0:
                # DMA-paced first block: one wide wave over the (still idle)
                # attention PSUM banks so each arriving chunk feeds 6 matmuls;
                # the 2 leftover v chains ride yps/sps (freed by the fast
                # v-copies, which are evacuated before the rope ops below).
                waves = [chains[0:6], chains[6:8]]
                wave_tags = [["pp", "pp", "ps", "ps", "yps", "sps"],
                             ["yps", "sps"]]
            else:
                waves = [[c] for c in chains]
                wave_tags = [["pp"]] * 8
            tagbufs = {"pp": 2, "ps": 2, "yps": 2, "sps": 1}
            for wv, wave in enumerate(waves):
                pss = [
                    psum.tile([128, 512], F32, tag=wave_tags[wv][j],
                              bufs=tagbufs[wave_tags[wv][j]],
                              name=f"p{tb}_{wv}{j}")
                    for j in range(len(wave))
                ]
                for ke in range(KE):
                    for j, (kind, idx) in enumerate(wave):
                        if kind == "qk":
                            nc.tensor.matmul(
                                pss[j],
                                lhsT=wq_sb[ke][:, idx * 128:(idx + 1) * 128],
                                rhs=xc[ke],
                                start=(ke == 0),
                                stop=(ke == KE - 1),
                            )
                        else:
                            nc.tensor.matmul(
                                pss[j][:, 0:HL * D],
                                lhsT=xc[ke][:, idx * 128:(idx + 1) * 128],
                                rhs=wq_sb[ke][:, 2 * HL * 128:],
                                start=(ke == 0),
                                stop=(ke == KE - 1),
                            )
                    yield
                # evacuate v chains first: they're cheap copies and free their
                # banks (yps/sps) for the next wave before the rope ops queue
                for j, (kind, idx) in sorted(enumerate(wave),
                                             key=lambda t: t[1][0] != "v"):
                    ps = pss[j]
                    if kind == "qk":
                        rb = idx
                        # rope: dst = t*cos + swap(t)*sin_signed, bf16 out
                        dst = (q_sb if rb < HL else k_sb)[b][rb % HL]
                        sl = bass.ds(soff, 512)
                        tsw = ropet.tile([128, 512], F32, tag="tsw", name=f"tsw{tb}{rb}")
                        tco = ropet.tile([128, 512], F32, tag="tco", name=f"tco{tb}{rb}")
                        nc.vector.tensor_mul(tsw[0:64, :], ps[64:128, :], sin_sb[0:64, sl])
                        nc.vector.tensor_mul(tsw[64:128, :], ps[0:64, :], sin_sb[64:128, sl])
                        nc.vector.tensor_mul(tco, ps, cos_sb[:, sl])
                        nc.vector.tensor_add(dst[:, sl], tco, tsw)
                    else:
                        blk = (soff // 128) + idx
                        nc.vector.tensor_copy(v_sb[b][:, blk, :], ps[:, 0:HL * D])

        def attn_panel(b, hl, p):
            """One attention q-panel (512 cols) for one head. Yields after each
            k-block (~645ns PE). One-block lookahead: scores(kb+1) is emitted
            before A@V(kb)/sums(kb) so exp latency hides behind PE work."""
            nkb = 4 * p + 4
            yps = psum.tile([128, 512], F32, tag="yps", bufs=2, name=f"yps{b}{hl}{p}")
            sps = psum.tile([128, 512], F32, tag="sps", bufs=1, name=f"sps{b}{hl}{p}")

            def av_sums(kb, at, qoff):
                nc.tensor.matmul(
                    yps[:, qoff:512],
                    lhsT=v_sb[b][:, kb, hl * D:(hl + 1) * D],
                    rhs=at[:, qoff:512],
                    start=(kb == 0),
                    stop=(kb == nkb - 1),
                )
                nc.tensor.matmul(
                    sps[:, qoff:512],
                    lhsT=ones_kk,
                    rhs=at[:, qoff:512],
                    start=(kb == 0),
                    stop=(kb == nkb - 1),
                )

            pending = None
            for kb in range(nkb):
                # kb's causally-valid q columns within the panel start at qoff;
                # kb=0 always has qoff=0 (start=True initializes all columns),
                # so later kbs may accumulate partial column ranges
                qoff = max(0, kb - 4 * p) * 128
                at = attnp.tile([128, 512], BF16, tag="attn", name=f"at{b}{hl}{p}{kb}")
                ps = psum.tile([128, 512], F32, tag="ps", bufs=2, name=f"sc{b}{hl}{p}{kb}")
                nc.tensor.matmul(
                    ps[:, 0:512 - qoff],
                    lhsT=k_sb[b][hl][:, kb * 128:(kb + 1) * 128],
                    rhs=q_sb[b][hl][:, p * 512 + qoff:(p + 1) * 512],
                    start=True,
                    stop=True,
                )
                nc.scalar.activation(
                    at[:, qoff:512],
                    ps[:, 0:512 - qoff],
                    mybir.ActivationFunctionType.Exp,
                    scale=SOFTMAX_SCALE,
                )
                if kb >= 4 * p:  # diagonal block: zero the k>q half
                    nc.vector.tensor_mul(
                        at[:, qoff:qoff + 128], at[:, qoff:qoff + 128], mask_sb
                    )
                if pending is not None:
                    av_sums(*pending)
                pending = (kb, at, qoff)
                yield
            av_sums(*pending)
            rb_sb = evacp.tile([128, 512], F32, tag="rb", name=f"rb{b}{hl}{p}")
            nc.vector.reciprocal_approx_fast(out=rb_sb, in_=sps)
            nc.vector.tensor_mul(y_sb[b][hl][:, p * 512:(p + 1) * 512], yps, rb_sb)
            yield

        def outproj_tkb(b, tkb, tags, writer="row"):
            """Out-proj of one 128-token block: 4 oc-chains [128 tok, 512 oc] =
            sum_hl y^T chunk @ w_out, DVE-evacuated into one full-row bf16
            tile. writer='row': ONE contiguous 512KB DMA on the gpsimd/SWDGE
            queue (keeps Sync free for x loads). writer='oc': four small Sync
            writes fired as each chunk lands (for the final tail, where Sync
            is idle and write latency is exposed). Yields per oc-chain."""
            tok0 = b * S + tkb * 128
            ot = outp.tile([128, E], BF16, tag="ot", bufs=2, name=f"ot{b}{tkb}")
            for oc in range(E // 512):
                tag = tags[oc % len(tags)]
                ops = psum.tile([128, 512], F32, tag=tag,
                                bufs=1 if tag == "ops" else 2,
                                name=f"o{b}{tkb}{oc}")
                for hl in range(HL):
                    nc.tensor.matmul(
                        ops,
                        lhsT=y_sb[b][hl][:, tkb * 128:(tkb + 1) * 128],
                        rhs=wo_sb[:, hl, oc * 512:(oc + 1) * 512],
                        start=(hl == 0),
                        stop=(hl == HL - 1),
                    )
                nc.vector.tensor_copy(ot[:, oc * 512:(oc + 1) * 512], ops)
                if writer == "oc":
                    nc.sync.dma_start(
                        out=out[tok0:tok0 + 128, oc * 512:(oc + 1) * 512],
                        in_=ot[:, oc * 512:(oc + 1) * 512],
                    )
                yield
            if writer == "row":
                nc.gpsimd.dma_start(out=out[tok0:tok0 + 128, :], in_=ot)

        # ============ scheduling machinery ==================================

        def chain_gens(gens):
            for g in gens:
                yield from g

        def gen_thunks(gen, n):
            """n single-step thunks + a drain thunk (runs trailing emissions
            after the last yield)."""
            it = iter(gen)

            def step():
                try:
                    next(it)
                except StopIteration:
                    pass

            def drain():
                for _ in it:
                    pass
            return [step] * n + [drain]

        def interleave(primary, filler):
            """Run primary thunks in order, spreading all filler thunks evenly
            between them; drain leftover filler at the end."""
            fsteps = deque(filler)
            ratio = (len(fsteps) / len(primary)) if primary else 0.0
            acc = 0.0
            for pstep in primary:
                pstep()
                acc += ratio
                while acc >= 1.0 and fsteps:
                    fsteps.popleft()()
                    acc -= 1.0
            while fsteps:
                fsteps.popleft()()

        def proj_steps(b, sb4):
            tb = b * (S // 512) + sb4
            n = 4 * KE if tb == 0 else 8 * KE
            return gen_thunks(proj_tb(b, sb4), n)

        def attn_steps(b, p):
            n = 2 * (4 * p + 4 + 1)
            return gen_thunks(
                chain_gens([attn_panel(b, 0, p), attn_panel(b, 1, p)]), n
            )

        def outproj_steps(b, p, tags=("ops", "pp"), writer="row"):
            gens = [outproj_tkb(b, tkb, tags, writer)
                    for tkb in range(4 * p, 4 * p + 4)]
            return gen_thunks(chain_gens(gens), 16)

        # ============ the interleaved schedule ==============================
        # Forward panel order: attn(b, p) needs only proj token-blocks <= p of
        # batch b, so projection of later blocks and out-proj of earlier panels
        # fill the PE while attention waits on exp/DVE. Out-proj of the last
        # panel rides inside the next batch's projection ("ops" tag only, since
        # "pp" is then busy with proj chains).
        for b in range(B):
            tail = outproj_steps(b - 1, 3, tags=("ops",)) if b > 0 else []
            interleave(proj_steps(b, 0) + proj_steps(b, 1), tail)
            open_tb(b, 2)
            interleave(attn_steps(b, 0), proj_steps(b, 2))
            open_tb(b, 3)
            if b + 1 < B:
                open_tb(b + 1, 0)   # prefetch next batch's x over this one's
            interleave(attn_steps(b, 1), proj_steps(b, 3))
            if b + 1 < B:
                open_tb(b + 1, 1)
            interleave(attn_steps(b, 2),
                       outproj_steps(b, 0) + outproj_steps(b, 1))
            interleave(attn_steps(b, 3), outproj_steps(b, 2))
        for t in outproj_steps(B - 1, 3, writer="oc"):
            t()


def build():
    nc = bacc.Bacc("TRN2", target_bir_lowering=False, debug=False)
    xT = nc.dram_tensor("xT", [E, NTOK], BF16, kind="ExternalInput").ap()
    wqkvT = nc.dram_tensor("wqkvT", [E, 3 * HL * D], BF16, kind="ExternalInput").ap()
    w_outT = nc.dram_tensor("w_outT", [HL * D, E], BF16, kind="ExternalInput").ap()
    out = nc.dram_tensor("out", [NTOK, E], BF16, kind="ExternalOutput").ap()

    cosT, sinS = _rope_tables()
    cos_d = nc.inline_tensor(cosT, name="cos_t").ap()
    sin_d = nc.inline_tensor(sinS, name="sin_t").ap()
    # maskT01[k, q] = 1 where k <= q (valid), else 0 — transposed-causal
    mask = np.triu(np.ones((128, 128), np.float32)).astype(ml_dtypes.bfloat16)
    mask_d = nc.inline_tensor(mask, name="maskT01").ap()

    with tile.TileContext(nc) as tc:
        _emit(nc, tc, xT, wqkvT, w_outT, out, cos_d, sin_d, mask_d)
    nc.compile()
    return nc


def make_in_maps(x, w_qkv, w_out):
    bf = ml_dtypes.bfloat16
    x2 = np.asarray(x, np.float32).reshape(NTOK, E)
    xT = np.ascontiguousarray(x2.astype(bf).T)                      # [E, NTOK]
    w_qkv = np.asarray(w_qkv, np.float32)
    w_out = np.asarray(w_out, np.float32)
    in_maps = []
    for c in range(NCORES):
        hs = [HL * c + j for j in range(HL)]
        rows = np.concatenate(
            [w_qkv[t * E + h * D:t * E + (h + 1) * D] for t in range(3) for h in hs]
        )                                                           # [768, E]
        wqkvT = np.ascontiguousarray(rows.astype(bf).T)             # [E, 768]
        w_outT = np.ascontiguousarray(
            w_out[:, c * HL * D:(c + 1) * HL * D].astype(bf).T      # [256, E]
        )
        in_maps.append({"xT": xT, "wqkvT": wqkvT, "w_outT": w_outT})
    return in_maps


_NC = None


def kernel(x, w_qkv, w_out):
    global _NC
    if _NC is None:
        _NC = build()
    in_maps = make_in_maps(x, w_qkv, w_out)
    res = run_bass_kernel_spmd(_NC, in_maps, core_ids=list(range(NCORES)))
    total = np.zeros((NTOK, E), np.float32)
    for r in res.results:
        total += r["out"].astype(np.float32)
    return total.reshape(B, S, E)
